# revision 17
# baseline (speedup 1.0000x reference)
"""Trainium2 Bass kernel for nn_Attention_8933531976242.

Multi-head self-attention (torch F.multi_head_attention_forward semantics):
  q = (X @ Wq.T + bq) * DH**-0.5 ; k = X @ Wk.T [+ bk] ; v = X @ Wv.T [+ bv]
  scores = q k^T + causal_mask ; key_padding -> NEG ; softmax ; ctx = p v
  out = ctx @ Wo.T + bo

Sharding (8 cores, Megatron column-parallel):
  Core c owns head-dim slice [128c, 128c+128) (2 heads of 16) for both
  batches: computes its q/k/v projections, attention for its 4 (b,h)
  pairs, and a partial output projection  ctx_c @ Wo[:, slice].T.
  The host sums the 8 partials (bf16) and adds the folded bias.

Design notes (empirically driven off NTFF hardware traces):
  - ALL matmuls run in bf16: on TRN2 hardware f32r streams 1 cyc/row only
    for full 128x128 stationary tiles; K=64 (scores) and M=65 (PV) tiles
    drop to 2 cyc/row. bf16 is 1 cyc/row for every shape.
  - bk is dropped entirely (softmax over s is invariant to the per-query
    constant q_t . bk); bv is folded into the host-side bias (softmax
    weights sum to 1, so bv contributes Wo @ bv to every output row).
  - key-padding is applied by ZEROING masked columns of v AND of the
    ones-column used for the denominator (exactly equivalent to the
    exp(NEG)=0 bias path), so the exp needs no per-chunk bias.
  - scores are computed TRANSPOSED: sT[s, t] = k_s . q_t with s on
    partitions; max-free softmax (scores bounded, |s| < ~8).
  - denominators come free from the PE: v is augmented with the kpm01
    column, so PV produces ctxT_aug [65, t] whose row 64 is
    sum_{s unmasked} p[s,t].
  - pair (0,0)'s scores+exp are PRECOMPUTED during the projection phase
    (interleaved with the rc4-7 / vtr tasks): the Act engine is otherwise
    idle there, and psum fits (2 proj banks + 2x2-bank score slabs).
    The probabilities persist in dedicated SBUF tiles (~35KB/partition).
  - rows whose causal prefix is fully key-padded are patched on the host
    from the key_padding_mask alone.
"""

import os
import sys
import numpy as np
from contextlib import ExitStack

for _p in ("/opt/trn_rl_repo", "/root/.axon_site/_ro/trn_rl_repo"):
    if os.path.isdir(_p) and _p not in sys.path:
        sys.path.append(_p)

T, B, E, H, DH = 2048, 2, 1024, 16, 64
SCALE = DH ** -0.5
NEG = float(np.finfo(np.float32).min)
NCORES = 8
R = T * B          # 4096 rows, batch-major: row = b*T + t
NTC = T // 512     # 4 t-chunks of 512 per (b,h) pair
NSC = T // 128     # 16 s-chunks of 128 per (b,h) pair


def ts(i, size):
    return slice(i * size, (i + 1) * size)


def build_nc():
    import concourse.bacc as bacc
    import concourse.tile as tile

    nc = bacc.Bacc("TRN2", target_bir_lowering=False, debug=False,
                   num_devices=NCORES)
    with tile.TileContext(nc) as tc:
        with ExitStack() as ctx:
            _trace_kernel(ctx, tc)
    nc.compile()
    return nc


def _trace_kernel(ctx, tc):
    import concourse.bass as bass
    import concourse.mybir as mybir

    nc = tc.nc
    f32 = mybir.dt.float32
    bf16 = mybir.dt.bfloat16
    Exp = mybir.ActivationFunctionType.Exp
    add_op = mybir.AluOpType.add
    mult_op = mybir.AluOpType.mult

    # ---------------- DRAM I/O ----------------
    xt = nc.dram_tensor("xt", [E, R], bf16, kind="ExternalInput").ap()
    wqt = nc.dram_tensor("wqt", [E, 128], bf16, kind="ExternalInput").ap()
    wkt = nc.dram_tensor("wkt", [E, 128], bf16, kind="ExternalInput").ap()
    wvt = nc.dram_tensor("wvt", [E, 128], bf16, kind="ExternalInput").ap()
    wot = nc.dram_tensor("wot", [128, E], bf16, kind="ExternalInput").ap()
    bqs = nc.dram_tensor("bqs", [128, 1], f32, kind="ExternalInput").ap()
    kpm = nc.dram_tensor("kpm", [128, B * NSC], f32, kind="ExternalInput").ap()
    caus = nc.dram_tensor("caus", [128, 128], bf16, kind="ExternalInput").ap()
    iden = nc.dram_tensor("iden", [128, 128], bf16, kind="ExternalInput").ap()
    # partial outputs ship bf16: host accumulates the 8 cores in f64
    outp = nc.dram_tensor("outp", [R, E], bf16, kind="ExternalOutput").ap()

    # ---------------- pools ----------------
    pw = ctx.enter_context(tc.tile_pool(name="weights", bufs=1))
    pbig = ctx.enter_context(tc.tile_pool(name="big", bufs=1))
    pxt = ctx.enter_context(tc.tile_pool(name="xtiles", bufs=3))
    pprob = ctx.enter_context(tc.tile_pool(name="probs", bufs=4))
    pprob0 = ctx.enter_context(tc.tile_pool(name="probs0", bufs=1))
    pctxsb = ctx.enter_context(tc.tile_pool(name="ctxsb", bufs=2))
    posb = ctx.enter_context(tc.tile_pool(name="osb", bufs=4))
    psmall = ctx.enter_context(tc.tile_pool(name="small", bufs=2))
    # PSUM budget (8 banks): projection phase runs 2 proj banks + 2x2-bank
    # score slabs (pair-(0,0) precompute); attention swaps the proj banks
    # for 4 ctx accumulators.
    pp_sc = ctx.enter_context(tc.tile_pool(name="pmm", bufs=2, space="PSUM"))
    pp_proj = tc.tile_pool(name="pproj", bufs=2, space="PSUM")
    pp_projh = pp_proj.__enter__()

    # ---------------- constants / weights ----------------
    def wtile(nm, src):
        w = pw.tile([128, 8 * 128], bf16, tag=nm, name=f"{nm}_sb")
        nc.sync.dma_start(w[:, :].rearrange("p (e m) -> p e m", e=8),
                          src[:, :].rearrange("(e p) m -> p e m", p=128))
        return [w[:, ts(e, 128)] for e in range(8)]

    # first projection's inputs stream first: wq, then the rc0 x-chunk
    wq_sb = wtile("wq", wqt)
    xtt0 = pxt.tile([128, 8 * 512], bf16, tag="xt", name="xt0")
    nc.sync.dma_start(xtt0[:, :].rearrange("p (e r) -> p e r", e=8),
                      xt[:, ts(0, 512)].rearrange("(e p) r -> p e r", p=128))
    wk_sb = wtile("wk", wkt)
    wv_sb = wtile("wv", wvt)
    bqs_sb = pw.tile([128, 1], f32, tag="bqs", name="bqs_sb")
    nc.sync.dma_start(bqs_sb[:, :], bqs[:, :])
    wot_sb = pw.tile([128, E], bf16, tag="wot", name="wot_sb")
    nc.sync.dma_start(wot_sb[:, :], wot[:, :])
    kpm_sb = pw.tile([128, B * NSC], f32, tag="kpm", name="kpm_sb")
    nc.sync.dma_start(kpm_sb[:, :], kpm[:, :])
    caus_sb = pw.tile([128, 128], bf16, tag="caus", name="caus_sb")
    nc.sync.dma_start(caus_sb[:, :], caus[:, :])
    iden_sb = pw.tile([128, 128], bf16, tag="iden", name="iden_sb")
    nc.sync.dma_start(iden_sb[:, :], iden[:, :])

    # ---------------- persistent activations ----------------
    qT = pbig.tile([128, R], bf16, tag="qT", name="qT")
    kT = pbig.tile([128, R], bf16, tag="kT", name="kT")
    vT = pbig.tile([128, R], bf16, tag="vT", name="vT")
    # v natural per s-chunk: [0:64] head0, [64] kpm01, [65:129] head1,
    # [129] kpm01 (kpm01 in place of ones => masked s contribute to
    # neither numerator nor denominator)
    v_sb = pbig.tile([128, 32 * 130], bf16, tag="v_sb", name="v_sb")
    v_cols = v_sb[:, :].rearrange("p (a c) -> p a c", c=130)
    kpm3 = kpm_sb[:, :].rearrange("p (a c) -> p a c", c=1)
    nc.vector.tensor_copy(v_cols[:, :, 64:65], kpm3[:, :, :])
    nc.vector.tensor_copy(v_cols[:, :, 129:130], kpm3[:, :, :])

    # ---------------- phase A: projections (qT/kT/vT) ----------------
    def emit_proj_rc(rc):
        if rc == 0:
            xtt = xtt0
        else:
            xtt = pxt.tile([128, 8 * 512], bf16, tag="xt", name=f"xt{rc}")
            nc.sync.dma_start(xtt[:, :].rearrange("p (e r) -> p e r", e=8),
                              xt[:, ts(rc, 512)].rearrange("(e p) r -> p e r",
                                                           p=128))
        xts = [xtt[:, ts(e, 512)] for e in range(8)]
        for wsb, dst, kind in ((wq_sb, qT, "q"), (wk_sb, kT, "k"),
                               (wv_sb, vT, "v")):
            ps = pp_projh.tile([128, 512], f32, tag="proj", name=f"proj{kind}{rc}")
            for e in range(8):
                nc.tensor.matmul(ps[:, :], lhsT=wsb[e], rhs=xts[e],
                                 start=(e == 0), stop=(e == 7))
            if kind == "q":
                nc.vector.tensor_scalar(dst[:, ts(rc, 512)], ps[:, :],
                                        SCALE, bqs_sb[:, 0:1],
                                        op0=mult_op, op1=add_op)
            else:
                nc.vector.tensor_copy(dst[:, ts(rc, 512)], ps[:, :])

    def emit_vtr(sc):
        pt = pp_projh.tile([128, 128], bf16, tag="proj", name=f"vtr{sc}")
        nc.tensor.transpose(pt[:, :], vT[:, ts(sc, 128)], iden_sb[:, :])
        # one 2-segment op: psum [128,(2,64)] -> v_sb cols [0:64] + [65:129],
        # multiplied by the per-s kpm01 mask (zeroes padded keys)
        dst = v_sb[:, 130 * sc: 130 * sc + 130].rearrange(
            "p (a c) -> p a c", a=2)[:, :, 0:64]
        src = pt[:, :].rearrange("p (a c) -> p a c", a=2)
        nc.vector.tensor_scalar(dst, src, kpm_sb[:, sc:sc + 1], None,
                                op0=mult_op)

    # warm the PE (HAM) during the prologue DMA wait: matmuls on a
    # zeroed scratch tile, result never read
    warm = pw.tile([128, 512], bf16, tag="warm", name="warm")
    nc.gpsimd.memset(warm[:, :], 0.0)
    for wi in range(24):
        wps = pp_projh.tile([128, 512], f32, tag="proj", name=f"warm{wi}")
        nc.tensor.matmul(wps[:, :], lhsT=warm[:, 0:128], rhs=warm[:, :],
                         start=True, stop=True)

    # ---------------- attention building blocks ----------------
    def emit_scores_exp(b, h, j, pj):
        """sT[s, t] = k_s . q_t for s-chunk j, exp'd into pj (sbuf)."""
        hp = slice(64 * h, 64 * h + 64)
        c0 = j // 4
        for half in range(c0 // 2, 2):
            t_lo = max(1024 * half, 128 * j)
            t_hi = 1024 * (half + 1)
            if t_lo >= t_hi:
                continue
            # slab columns live at t - 1024*half so every matmul write
            # stays 512-aligned within its psum bank
            s_off = t_lo - 1024 * half
            sp = pp_sc.tile([128, 1024], f32, tag="mm", name=f"s{b}{h}{j}{half}")
            for c in range(2 * half, 2 * half + 2):
                lo = max(512 * c, t_lo)
                hi = 512 * (c + 1)
                if lo >= hi:
                    continue
                nc.tensor.matmul(
                    sp[:, lo - 1024 * half: hi - 1024 * half],
                    lhsT=kT[hp, b * T + 128 * j: b * T + 128 * (j + 1)],
                    rhs=qT[hp, b * T + lo: b * T + hi],
                    start=True, stop=True)
            nc.scalar.activation(
                pj[:, t_lo - 128 * j: t_hi - 128 * j],
                sp[:, s_off: 1024], Exp, bias=0.0, scale=1.0)
            if t_lo == 128 * j:
                # zero the upper triangle of the diagonal block after exp
                # (multiplicative template keeps the scores->exp chain free)
                nc.vector.tensor_tensor(pj[:, 0:128], pj[:, 0:128],
                                        caus_sb[:, :], op=mult_op)

    out_pending = []

    def emit_pv(b, h, j, pj, ctx_ps, ctxsb):
        """PV accumulate for s-chunk j; on completing a t-chunk, normalize
        it into ctxsb and (for h==1) queue its output projection."""
        c0 = j // 4
        for c in list(range(c0 + 1, NTC)) + [c0]:
            lo = max(512 * c, 128 * j)
            hi = 512 * (c + 1)
            nc.tensor.matmul(
                ctx_ps[c][:, lo - 512 * c: 512],
                lhsT=v_sb[:, 130 * (b * NSC + j) + 65 * h:
                          130 * (b * NSC + j) + 65 * h + 65],
                rhs=pj[:, lo - 128 * j: hi - 128 * j],
                start=(j == 0), stop=(j == 4 * c + 3),
                skip_group_check=True)
        if j % 4 == 3:
            c = j // 4
            hp = slice(64 * h, 64 * h + 64)
            den = psmall.tile([1, 512], f32, tag="den", name=f"d{b}{h}{c}")
            nc.vector.tensor_scalar_max(den[:, :], ctx_ps[c][64:65, :], 1e-30)
            rec = psmall.tile([1, 512], f32, tag="rec", name=f"r{b}{h}{c}")
            nc.vector.reciprocal_approx_fast(rec[:, :], den[:, :])
            rm = psmall.tile([64, 512], f32, tag="rm", name=f"rm{b}{h}{c}")
            nc.gpsimd.partition_broadcast(rm[:, :], rec[:, :], channels=64)
            nc.vector.tensor_tensor(ctxsb[hp, ts(c, 512)],
                                    ctx_ps[c][0:64, :], rm[:, :], op=mult_op)
            if h == 1:
                for i in range(4 * c, 4 * c + 4):
                    out_pending.append((b, i))

    def emit_outproj_chunk():
        """Project one queued 128-row chunk: ctx rows @ Wo_slice.T -> DMA.
        Deferred by at least one pipeline item so the den->rec->broadcast
        chain that gates it overlaps the next item's score matmuls instead
        of stalling the in-order PE queue."""
        if not out_pending:
            return
        b, i = out_pending.pop(0)
        ctxsb = ctxsbs[b]
        osb = posb.tile([128, 1024], bf16, tag="osb", name=f"ob{b}{i}")
        for nch in range(2):
            po = pp_ctx.tile([128, 512], f32, tag="ctx",
                             name=f"o{b}{i}{nch}")
            nc.tensor.matmul(po[:, :],
                             lhsT=ctxsb[:, ts(i, 128)],
                             rhs=wot_sb[:, ts(nch, 512)],
                             start=True, stop=True)
            # alternate psum->sbuf copies between DVE and Act: the
            # copies cluster where exp work is sparse (esp. the tail)
            if (i + nch) % 2 == 0:
                nc.vector.tensor_copy(osb[:, ts(nch, 512)], po[:, :])
            else:
                nc.scalar.copy(osb[:, ts(nch, 512)], po[:, :])
        nc.sync.dma_start(
            outp[b * T + 128 * i: b * T + 128 * (i + 1), :], osb[:, :])

    # ---------------- phase A + pair-(0,0) scores precompute ----------------
    for rc in range(4):
        emit_proj_rc(rc)
    for sc in range(16):
        emit_vtr(sc)
    # remaining projection/transpose work, drained between precompute items
    tasks = [("rc", rc) for rc in range(4, 8)] + \
            [("vtr", sc) for sc in range(16, 32)]
    pj0 = {}
    for j in range(NSC):
        for h in range(2):
            pj = pprob0.tile([128, T - 128 * j], bf16, tag=f"p0{h}_{j}",
                             name=f"p0{h}_{j}")
            pj0[(h, j)] = pj
            emit_scores_exp(0, h, j, pj)
            if tasks:
                kind, a = tasks.pop(0)
                if kind == "rc":
                    emit_proj_rc(a)
                else:
                    emit_vtr(a)
    while tasks:
        kind, a = tasks.pop(0)
        if kind == "rc":
            emit_proj_rc(a)
        else:
            emit_vtr(a)
    pp_proj.__exit__(None, None, None)
    pp_ctx = ctx.enter_context(tc.tile_pool(name="pctx", bufs=4, space="PSUM"))

    # ---------------- phase B/C: attention + output projection ----------------
    # software-pipelined across ALL (b, h, j): scores(i+1) is emitted
    # before PV(i) so the PE always has independent matmuls queued ahead
    # of the exp(i) wait, including across pair boundaries.  Pair (0,0)
    # items reuse the precomputed probabilities (no scores/exp emitted).
    ctxsbs = {0: pctxsb.tile([128, T], bf16, tag="ctxsb", name="ctx0"),
              1: pctxsb.tile([128, T], bf16, tag="ctxsb", name="ctx1")}
    items = [(b, h, j) for b in range(B) for h in range(2)
             for j in range(NSC)]
    ctx_tiles = {}
    prev = None
    for (b, h, j) in items:
        if j == 0:
            ctx_tiles[(b, h)] = [pp_ctx.tile([65, 512], f32, tag="ctx",
                                             name=f"ctxp{b}{h}{c}")
                                 for c in range(NTC)]
        if b == 0:
            pj = pj0[(h, j)]
        else:
            pj = pprob.tile([128, T - 128 * j], bf16, tag="probs",
                            name=f"p{b}{h}{j}")
            emit_scores_exp(b, h, j, pj)
        emit_outproj_chunk()
        if prev is not None:
            pb, ph, pjj, ppj = prev
            emit_pv(pb, ph, pjj, ppj, ctx_tiles[(pb, ph)], ctxsbs[pb])
        prev = (b, h, j, pj)
    pb, ph, pjj, ppj = prev
    emit_pv(pb, ph, pjj, ppj, ctx_tiles[(pb, ph)], ctxsbs[pb])
    while out_pending:
        emit_outproj_chunk()


# ---------------------------------------------------------------------------
# host side
# ---------------------------------------------------------------------------
_NC_CACHE = {}


def _get_nc():
    if "nc" not in _NC_CACHE:
        _NC_CACHE["nc"] = build_nc()
    return _NC_CACHE["nc"]


def make_in_maps(query, key_padding_mask, Wq, bq, Wk, Wv, Wo):
    import ml_dtypes
    f32 = np.float32
    pnp = ml_dtypes.bfloat16
    # batch-major rows: row = b*T + t
    Xbm = np.ascontiguousarray(query.transpose(1, 0, 2).reshape(R, E))
    XT = np.ascontiguousarray(Xbm.T)                       # [E, R]
    kpm01 = (~key_padding_mask).astype(f32)                # [B, T] 1=keep
    kpm_arr = np.ascontiguousarray(
        kpm01.reshape(B, NSC, 128).transpose(2, 0, 1).reshape(128, B * NSC))
    caus = (np.arange(128)[:, None] <= np.arange(128)[None, :]).astype(f32)
    iden = np.eye(128, dtype=f32)
    in_maps = []
    xt_bf = np.ascontiguousarray(XT.astype(pnp))
    for c in range(NCORES):
        sl = slice(128 * c, 128 * (c + 1))
        in_maps.append({
            "xt": xt_bf,
            "wqt": np.ascontiguousarray(Wq[sl, :].T.astype(pnp)),
            "wkt": np.ascontiguousarray(Wk[sl, :].T.astype(pnp)),
            "wvt": np.ascontiguousarray(Wv[sl, :].T.astype(pnp)),
            "wot": np.ascontiguousarray(Wo[:, sl].T.astype(pnp)),
            "bqs": (bq[sl] * SCALE).astype(f32).reshape(128, 1),
            "kpm": kpm_arr,
            "caus": caus.astype(pnp),
            "iden": iden.astype(pnp),
        })
    return in_maps


def combine_outputs(parts, query, key_padding_mask, Wv, bv, Wo, bo):
    acc = np.zeros((R, E), dtype=np.float64)
    for p in parts:
        acc += np.asarray(p).astype(np.float64)
    # bv is not applied on device: softmax weights sum to 1, so v's bias
    # contributes the constant row Wo @ bv to every output
    out_bm = acc + (bo + Wo @ bv).astype(np.float64)
    out = out_bm.reshape(B, T, E).transpose(1, 0, 2).astype(np.float32)
    # degenerate rows: causal prefix fully key-padded -> uniform softmax
    # over ALL T columns in the reference
    for b in range(B):
        pref = np.cumsum(~key_padding_mask[b]) == 0
        degen = np.nonzero(pref)[0]
        if len(degen):
            mean_x = query[:, b, :].mean(axis=0)
            ctx_deg = mean_x @ Wv.T + bv
            row = (ctx_deg @ Wo.T + bo).astype(np.float32)
            out[degen, b, :] = row
    return np.ascontiguousarray(out)


def _ensure_ntff_hook():
    """The agent image's antenv lacks axon_hooks; synthesize it so
    run_bass_kernel_spmd(trace=True) can reach the NTFF profiler."""
    try:
        import antenv.axon_hooks  # noqa: F401
        return
    except ImportError:
        pass
    import types
    import antenv
    from trn_agent_boot.trn_boot import _ntff_profile_via_ctypes
    hook = _ntff_profile_via_ctypes("/opt/axon/libaxon_pjrt.so")
    mod = types.ModuleType("antenv.axon_hooks")
    mod._hook = hook
    mod.get_axon_ntff_profile_hook = lambda: mod._hook
    mod.set_axon_ntff_profile_hook = lambda h: setattr(mod, "_hook", h)
    sys.modules["antenv.axon_hooks"] = mod
    antenv.axon_hooks = mod


def kernel(query, key_padding_mask, attn_mask, Wq, bq, Wk, bk, Wv, bv, Wo, bo,
           _profile=False):
    from concourse.bass_utils import run_bass_kernel_spmd

    if _profile:
        try:
            _ensure_ntff_hook()
        except Exception as e:  # profiling is best-effort
            print(f"ntff hook unavailable: {e}")

    query = np.asarray(query, dtype=np.float32)
    key_padding_mask = np.asarray(key_padding_mask).astype(bool)
    in_maps = make_in_maps(query, key_padding_mask,
                           np.asarray(Wq, np.float32), np.asarray(bq, np.float32),
                           np.asarray(Wk, np.float32),
                           np.asarray(Wv, np.float32),
                           np.asarray(Wo, np.float32))
    nc = _get_nc()
    res = run_bass_kernel_spmd(nc, in_maps, core_ids=list(range(NCORES)),
                               trace=_profile)
    parts = [res.results[c]["outp"] for c in range(NCORES)]
    out = combine_outputs(parts, query, key_padding_mask,
                          np.asarray(Wv, np.float32), np.asarray(bv, np.float32),
                          np.asarray(Wo, np.float32), np.asarray(bo, np.float32))
    if _profile:
        return out, res
    return out


# revision 19
# speedup vs baseline: 1.0887x; 1.0887x over previous
"""Trainium2 Bass kernel for nn_Attention_8933531976242.

Multi-head self-attention (torch F.multi_head_attention_forward semantics):
  q = (X @ Wq.T + bq) * DH**-0.5 ; k = X @ Wk.T [+ bk] ; v = X @ Wv.T [+ bv]
  scores = q k^T + causal_mask ; key_padding -> NEG ; softmax ; ctx = p v
  out = ctx @ Wo.T + bo

Sharding (8 cores, Megatron column-parallel):
  Core c owns head-dim slice [128c, 128c+128) (2 heads of 16) for both
  batches: computes its q/k/v projections, attention for its 4 (b,h)
  pairs, and a partial output projection  ctx_c @ Wo[:, slice].T.
  The host sums the 8 partials (bf16) and adds the folded bias.

Design notes (empirically driven off NTFF hardware traces):
  - ALL matmuls run in bf16: on TRN2 hardware f32r streams 1 cyc/row only
    for full 128x128 stationary tiles; K=64 (scores) and M=65 (PV) tiles
    drop to 2 cyc/row. bf16 is 1 cyc/row for every shape.
  - bk is dropped entirely (softmax over s is invariant to the per-query
    constant q_t . bk); bv is folded into the host-side bias (softmax
    weights sum to 1, so bv contributes Wo @ bv to every output row).
  - key-padding is applied by ZEROING masked columns of v AND of the
    ones-column used for the denominator (exactly equivalent to the
    exp(NEG)=0 bias path), so the exp needs no per-chunk bias.
  - scores are computed TRANSPOSED: sT[s, t] = k_s . q_t with s on
    partitions; max-free softmax (scores bounded, |s| < ~8).
  - denominators come free from the PE: v is augmented with the kpm01
    column, so PV produces ctxT_aug [65, t] whose row 64 is
    sum_{s unmasked} p[s,t].
  - pair (0,0)'s scores+exp are PRECOMPUTED during the projection phase
    (interleaved with the rc4-7 / vtr tasks): the Act engine is otherwise
    idle there, and psum fits (2 proj banks + 2x2-bank score slabs).
    The probabilities persist in dedicated SBUF tiles (~35KB/partition).
  - rows whose causal prefix is fully key-padded are patched on the host
    from the key_padding_mask alone.
"""

import os
import sys
import numpy as np
from contextlib import ExitStack

for _p in ("/opt/trn_rl_repo", "/root/.axon_site/_ro/trn_rl_repo"):
    if os.path.isdir(_p) and _p not in sys.path:
        sys.path.append(_p)

T, B, E, H, DH = 2048, 2, 1024, 16, 64
SCALE = DH ** -0.5
NEG = float(np.finfo(np.float32).min)
NCORES = 8
R = T * B          # 4096 rows, batch-major: row = b*T + t
NTC = T // 512     # 4 t-chunks of 512 per (b,h) pair
NSC = T // 128     # 16 s-chunks of 128 per (b,h) pair


def ts(i, size):
    return slice(i * size, (i + 1) * size)


def build_nc():
    import concourse.bacc as bacc
    import concourse.tile as tile

    nc = bacc.Bacc("TRN2", target_bir_lowering=False, debug=False,
                   num_devices=NCORES)
    with tile.TileContext(nc) as tc:
        with ExitStack() as ctx:
            _trace_kernel(ctx, tc)
    nc.compile()
    return nc


def _trace_kernel(ctx, tc):
    import concourse.bass as bass
    import concourse.mybir as mybir

    nc = tc.nc
    f32 = mybir.dt.float32
    bf16 = mybir.dt.bfloat16
    Exp = mybir.ActivationFunctionType.Exp
    add_op = mybir.AluOpType.add
    mult_op = mybir.AluOpType.mult

    # ---------------- DRAM I/O ----------------
    xt = nc.dram_tensor("xt", [E, R], bf16, kind="ExternalInput").ap()
    wqt = nc.dram_tensor("wqt", [E, 128], bf16, kind="ExternalInput").ap()
    wkt = nc.dram_tensor("wkt", [E, 128], bf16, kind="ExternalInput").ap()
    wvt = nc.dram_tensor("wvt", [E, 128], bf16, kind="ExternalInput").ap()
    wot = nc.dram_tensor("wot", [128, E], bf16, kind="ExternalInput").ap()
    bqs = nc.dram_tensor("bqs", [128, 1], f32, kind="ExternalInput").ap()
    kpm = nc.dram_tensor("kpm", [128, B * NSC], f32, kind="ExternalInput").ap()
    caus = nc.dram_tensor("caus", [128, 128], bf16, kind="ExternalInput").ap()
    iden = nc.dram_tensor("iden", [128, 128], bf16, kind="ExternalInput").ap()
    # partial outputs ship bf16: host accumulates the 8 cores in f64
    outp = nc.dram_tensor("outp", [R, E], bf16, kind="ExternalOutput").ap()

    # ---------------- pools ----------------
    pw = ctx.enter_context(tc.tile_pool(name="weights", bufs=1))
    pbig = ctx.enter_context(tc.tile_pool(name="big", bufs=1))
    pxt = ctx.enter_context(tc.tile_pool(name="xtiles", bufs=3))
    pprob = ctx.enter_context(tc.tile_pool(name="probs", bufs=4))
    pprob0 = ctx.enter_context(tc.tile_pool(name="probs0", bufs=1))
    pctxsb = ctx.enter_context(tc.tile_pool(name="ctxsb", bufs=2))
    posb = ctx.enter_context(tc.tile_pool(name="osb", bufs=4))
    psmall = ctx.enter_context(tc.tile_pool(name="small", bufs=2))
    # PSUM budget (8 banks): projection phase runs 2 proj banks + 2x2-bank
    # score slabs (pair-(0,0) precompute); attention swaps the proj banks
    # for 4 ctx accumulators.
    pp_sc = ctx.enter_context(tc.tile_pool(name="pmm", bufs=2, space="PSUM"))
    pp_proj = tc.tile_pool(name="pproj", bufs=2, space="PSUM")
    pp_projh = pp_proj.__enter__()

    # ---------------- constants / weights ----------------
    def wtile(nm, src):
        w = pw.tile([128, 8 * 128], bf16, tag=nm, name=f"{nm}_sb")
        nc.sync.dma_start(w[:, :].rearrange("p (e m) -> p e m", e=8),
                          src[:, :].rearrange("(e p) m -> p e m", p=128))
        return [w[:, ts(e, 128)] for e in range(8)]

    # first projection's inputs stream first: wq, then the rc0 x-chunk
    wq_sb = wtile("wq", wqt)
    xtt0 = pxt.tile([128, 8 * 512], bf16, tag="xt", name="xt0")
    nc.sync.dma_start(xtt0[:, :].rearrange("p (e r) -> p e r", e=8),
                      xt[:, ts(0, 512)].rearrange("(e p) r -> p e r", p=128))
    wk_sb = wtile("wk", wkt)
    wv_sb = wtile("wv", wvt)
    bqs_sb = pw.tile([128, 1], f32, tag="bqs", name="bqs_sb")
    nc.sync.dma_start(bqs_sb[:, :], bqs[:, :])
    wot_sb = pw.tile([128, E], bf16, tag="wot", name="wot_sb")
    nc.sync.dma_start(wot_sb[:, :], wot[:, :])
    kpm_sb = pw.tile([128, B * NSC], f32, tag="kpm", name="kpm_sb")
    nc.sync.dma_start(kpm_sb[:, :], kpm[:, :])
    caus_sb = pw.tile([128, 128], bf16, tag="caus", name="caus_sb")
    nc.sync.dma_start(caus_sb[:, :], caus[:, :])
    iden_sb = pw.tile([128, 128], bf16, tag="iden", name="iden_sb")
    nc.sync.dma_start(iden_sb[:, :], iden[:, :])

    # ---------------- persistent activations ----------------
    qT = pbig.tile([128, R], bf16, tag="qT", name="qT")
    kT = pbig.tile([128, R], bf16, tag="kT", name="kT")
    vT = pbig.tile([128, R], bf16, tag="vT", name="vT")
    # v natural per s-chunk: [0:64] head0, [64] kpm01, [65:129] head1,
    # [129] kpm01 (kpm01 in place of ones => masked s contribute to
    # neither numerator nor denominator)
    v_sb = pbig.tile([128, 32 * 130], bf16, tag="v_sb", name="v_sb")
    v_cols = v_sb[:, :].rearrange("p (a c) -> p a c", c=130)
    kpm3 = kpm_sb[:, :].rearrange("p (a c) -> p a c", c=1)
    nc.vector.tensor_copy(v_cols[:, :, 64:65], kpm3[:, :, :])
    nc.vector.tensor_copy(v_cols[:, :, 129:130], kpm3[:, :, :])

    # ---------------- phase A: projections (qT/kT/vT) ----------------
    def emit_proj_rc(rc):
        if rc == 0:
            xtt = xtt0
        else:
            xtt = pxt.tile([128, 8 * 512], bf16, tag="xt", name=f"xt{rc}")
            nc.sync.dma_start(xtt[:, :].rearrange("p (e r) -> p e r", e=8),
                              xt[:, ts(rc, 512)].rearrange("(e p) r -> p e r",
                                                           p=128))
        xts = [xtt[:, ts(e, 512)] for e in range(8)]
        for wsb, dst, kind in ((wq_sb, qT, "q"), (wk_sb, kT, "k"),
                               (wv_sb, vT, "v")):
            ps = pp_projh.tile([128, 512], f32, tag="proj", name=f"proj{kind}{rc}")
            for e in range(8):
                nc.tensor.matmul(ps[:, :], lhsT=wsb[e], rhs=xts[e],
                                 start=(e == 0), stop=(e == 7))
            if kind == "q":
                nc.vector.tensor_scalar(dst[:, ts(rc, 512)], ps[:, :],
                                        SCALE, bqs_sb[:, 0:1],
                                        op0=mult_op, op1=add_op)
            else:
                nc.vector.tensor_copy(dst[:, ts(rc, 512)], ps[:, :])

    def emit_vtr(sc):
        pt = pp_projh.tile([128, 128], bf16, tag="proj", name=f"vtr{sc}")
        nc.tensor.transpose(pt[:, :], vT[:, ts(sc, 128)], iden_sb[:, :])
        # one 2-segment op: psum [128,(2,64)] -> v_sb cols [0:64] + [65:129],
        # multiplied by the per-s kpm01 mask (zeroes padded keys)
        dst = v_sb[:, 130 * sc: 130 * sc + 130].rearrange(
            "p (a c) -> p a c", a=2)[:, :, 0:64]
        src = pt[:, :].rearrange("p (a c) -> p a c", a=2)
        nc.vector.tensor_scalar(dst, src, kpm_sb[:, sc:sc + 1], None,
                                op0=mult_op)

    # warm the PE (HAM) during the prologue DMA wait: matmuls on a
    # zeroed scratch tile, result never read
    warm = pw.tile([128, 512], bf16, tag="warm", name="warm")
    nc.gpsimd.memset(warm[:, :], 0.0)
    for wi in range(24):
        wps = pp_projh.tile([128, 512], f32, tag="proj", name=f"warm{wi}")
        nc.tensor.matmul(wps[:, :], lhsT=warm[:, 0:128], rhs=warm[:, :],
                         start=True, stop=True)

    # ---------------- attention building blocks ----------------
    def emit_scores_exp(b, h, j, pj):
        """sT[s, t] = k_s . q_t for s-chunk j, exp'd into pj (sbuf)."""
        hp = slice(64 * h, 64 * h + 64)
        c0 = j // 4
        for half in range(c0 // 2, 2):
            t_lo = max(1024 * half, 128 * j)
            t_hi = 1024 * (half + 1)
            if t_lo >= t_hi:
                continue
            # slab columns live at t - 1024*half so every matmul write
            # stays 512-aligned within its psum bank
            s_off = t_lo - 1024 * half
            sp = pp_sc.tile([128, 1024], f32, tag="mm", name=f"s{b}{h}{j}{half}")
            for c in range(2 * half, 2 * half + 2):
                lo = max(512 * c, t_lo)
                hi = 512 * (c + 1)
                if lo >= hi:
                    continue
                nc.tensor.matmul(
                    sp[:, lo - 1024 * half: hi - 1024 * half],
                    lhsT=kT[hp, b * T + 128 * j: b * T + 128 * (j + 1)],
                    rhs=qT[hp, b * T + lo: b * T + hi],
                    start=True, stop=True)
            nc.scalar.activation(
                pj[:, t_lo - 128 * j: t_hi - 128 * j],
                sp[:, s_off: 1024], Exp, bias=0.0, scale=1.0)
            if t_lo == 128 * j:
                # zero the upper triangle of the diagonal block after exp
                # (multiplicative template keeps the scores->exp chain free)
                nc.vector.tensor_tensor(pj[:, 0:128], pj[:, 0:128],
                                        caus_sb[:, :], op=mult_op)

    def emit_pv(b, h, j, pj, ctx_ps, ctxsb):
        """PV accumulate for s-chunk j; on completing a t-chunk, normalize
        it into ctxsb and (for h==1) emit its output projection."""
        c0 = j // 4
        for c in list(range(c0 + 1, NTC)) + [c0]:
            lo = max(512 * c, 128 * j)
            hi = 512 * (c + 1)
            nc.tensor.matmul(
                ctx_ps[c][:, lo - 512 * c: 512],
                lhsT=v_sb[:, 130 * (b * NSC + j) + 65 * h:
                          130 * (b * NSC + j) + 65 * h + 65],
                rhs=pj[:, lo - 128 * j: hi - 128 * j],
                start=(j == 0), stop=(j == 4 * c + 3),
                skip_group_check=True)
        if j % 4 == 3:
            c = j // 4
            hp = slice(64 * h, 64 * h + 64)
            den = psmall.tile([1, 512], f32, tag="den", name=f"d{b}{h}{c}")
            nc.vector.tensor_scalar_max(den[:, :], ctx_ps[c][64:65, :], 1e-30)
            rec = psmall.tile([1, 512], f32, tag="rec", name=f"r{b}{h}{c}")
            nc.vector.reciprocal_approx_fast(rec[:, :], den[:, :])
            rm = psmall.tile([64, 512], f32, tag="rm", name=f"rm{b}{h}{c}")
            nc.gpsimd.partition_broadcast(rm[:, :], rec[:, :], channels=64)
            nc.vector.tensor_tensor(ctxsb[hp, ts(c, 512)],
                                    ctx_ps[c][0:64, :], rm[:, :], op=mult_op)
            if h == 1:
                emit_outproj(b, ctxsb, c)

    def emit_outproj(b, ctxsb, c):
        """out rows [512c, 512c+512) of batch b: ctx_c @ Wo_slice.T."""
        for i in range(4 * c, 4 * c + 4):
            osb = posb.tile([128, 1024], bf16, tag="osb", name=f"ob{b}{i}")
            for nch in range(2):
                po = pp_ctx.tile([128, 512], f32, tag="ctx",
                                 name=f"o{b}{i}{nch}")
                nc.tensor.matmul(po[:, :],
                                 lhsT=ctxsb[:, ts(i, 128)],
                                 rhs=wot_sb[:, ts(nch, 512)],
                                 start=True, stop=True)
                # alternate psum->sbuf copies between DVE and Act: the
                # copies cluster where exp work is sparse (esp. the tail)
                if (i + nch) % 2 == 0:
                    nc.vector.tensor_copy(osb[:, ts(nch, 512)], po[:, :])
                else:
                    nc.scalar.copy(osb[:, ts(nch, 512)], po[:, :])
            nc.sync.dma_start(
                outp[b * T + 128 * i: b * T + 128 * (i + 1), :], osb[:, :])

    # ---------------- phase A + pair-(0,0) scores precompute ----------------
    for rc in range(4):
        emit_proj_rc(rc)
    for sc in range(16):
        emit_vtr(sc)
    # remaining projection/transpose work, drained between precompute items
    tasks = [("rc", rc) for rc in range(4, 8)] + \
            [("vtr", sc) for sc in range(16, 32)]
    pj0 = {}
    for j in range(NSC):
        for h in range(2):
            pj = pprob0.tile([128, T - 128 * j], bf16, tag=f"p0{h}_{j}",
                             name=f"p0{h}_{j}")
            pj0[(h, j)] = pj
            emit_scores_exp(0, h, j, pj)
            if tasks:
                kind, a = tasks.pop(0)
                if kind == "rc":
                    emit_proj_rc(a)
                else:
                    emit_vtr(a)
    while tasks:
        kind, a = tasks.pop(0)
        if kind == "rc":
            emit_proj_rc(a)
        else:
            emit_vtr(a)
    # batch-1 pairs' small s-chunks (j>=8): worst overhead-to-work ratio
    # in the attention lockstep; their q/k deps (rc4-7) are done by now
    pj1 = {}
    for j in range(8, NSC):
        for h in range(2):
            pj = pprob0.tile([128, T - 128 * j], bf16, tag=f"p1{h}_{j}",
                             name=f"p1{h}_{j}")
            pj1[(h, j)] = pj
            emit_scores_exp(1, h, j, pj)
    pp_proj.__exit__(None, None, None)
    pp_ctx = ctx.enter_context(tc.tile_pool(name="pctx", bufs=4, space="PSUM"))

    # ---------------- phase B/C: attention + output projection ----------------
    # software-pipelined across ALL (b, h, j): scores(i+1) is emitted
    # before PV(i) so the PE always has independent matmuls queued ahead
    # of the exp(i) wait, including across pair boundaries.  Pair (0,0)
    # items reuse the precomputed probabilities (no scores/exp emitted).
    ctxsbs = {0: pctxsb.tile([128, T], bf16, tag="ctxsb", name="ctx0"),
              1: pctxsb.tile([128, T], bf16, tag="ctxsb", name="ctx1")}
    items = [(b, h, j) for b in range(B) for h in range(2)
             for j in range(NSC)]
    ctx_tiles = {}
    prev = None
    for (b, h, j) in items:
        if j == 0:
            ctx_tiles[(b, h)] = [pp_ctx.tile([65, 512], f32, tag="ctx",
                                             name=f"ctxp{b}{h}{c}")
                                 for c in range(NTC)]
        if b == 0:
            pj = pj0[(h, j)]
        elif j >= 8:
            pj = pj1[(h, j)]
        else:
            pj = pprob.tile([128, T - 128 * j], bf16, tag="probs",
                            name=f"p{b}{h}{j}")
            emit_scores_exp(b, h, j, pj)
        if prev is not None:
            pb, ph, pjj, ppj = prev
            emit_pv(pb, ph, pjj, ppj, ctx_tiles[(pb, ph)], ctxsbs[pb])
        prev = (b, h, j, pj)
    pb, ph, pjj, ppj = prev
    emit_pv(pb, ph, pjj, ppj, ctx_tiles[(pb, ph)], ctxsbs[pb])


# ---------------------------------------------------------------------------
# host side
# ---------------------------------------------------------------------------
_NC_CACHE = {}


def _get_nc():
    if "nc" not in _NC_CACHE:
        _NC_CACHE["nc"] = build_nc()
    return _NC_CACHE["nc"]


def make_in_maps(query, key_padding_mask, Wq, bq, Wk, Wv, Wo):
    import ml_dtypes
    f32 = np.float32
    pnp = ml_dtypes.bfloat16
    # batch-major rows: row = b*T + t
    Xbm = np.ascontiguousarray(query.transpose(1, 0, 2).reshape(R, E))
    XT = np.ascontiguousarray(Xbm.T)                       # [E, R]
    kpm01 = (~key_padding_mask).astype(f32)                # [B, T] 1=keep
    kpm_arr = np.ascontiguousarray(
        kpm01.reshape(B, NSC, 128).transpose(2, 0, 1).reshape(128, B * NSC))
    caus = (np.arange(128)[:, None] <= np.arange(128)[None, :]).astype(f32)
    iden = np.eye(128, dtype=f32)
    in_maps = []
    xt_bf = np.ascontiguousarray(XT.astype(pnp))
    for c in range(NCORES):
        sl = slice(128 * c, 128 * (c + 1))
        in_maps.append({
            "xt": xt_bf,
            "wqt": np.ascontiguousarray(Wq[sl, :].T.astype(pnp)),
            "wkt": np.ascontiguousarray(Wk[sl, :].T.astype(pnp)),
            "wvt": np.ascontiguousarray(Wv[sl, :].T.astype(pnp)),
            "wot": np.ascontiguousarray(Wo[:, sl].T.astype(pnp)),
            "bqs": (bq[sl] * SCALE).astype(f32).reshape(128, 1),
            "kpm": kpm_arr,
            "caus": caus.astype(pnp),
            "iden": iden.astype(pnp),
        })
    return in_maps


def combine_outputs(parts, query, key_padding_mask, Wv, bv, Wo, bo):
    acc = np.zeros((R, E), dtype=np.float64)
    for p in parts:
        acc += np.asarray(p).astype(np.float64)
    # bv is not applied on device: softmax weights sum to 1, so v's bias
    # contributes the constant row Wo @ bv to every output
    out_bm = acc + (bo + Wo @ bv).astype(np.float64)
    out = out_bm.reshape(B, T, E).transpose(1, 0, 2).astype(np.float32)
    # degenerate rows: causal prefix fully key-padded -> uniform softmax
    # over ALL T columns in the reference
    for b in range(B):
        pref = np.cumsum(~key_padding_mask[b]) == 0
        degen = np.nonzero(pref)[0]
        if len(degen):
            mean_x = query[:, b, :].mean(axis=0)
            ctx_deg = mean_x @ Wv.T + bv
            row = (ctx_deg @ Wo.T + bo).astype(np.float32)
            out[degen, b, :] = row
    return np.ascontiguousarray(out)


def _ensure_ntff_hook():
    """The agent image's antenv lacks axon_hooks; synthesize it so
    run_bass_kernel_spmd(trace=True) can reach the NTFF profiler."""
    try:
        import antenv.axon_hooks  # noqa: F401
        return
    except ImportError:
        pass
    import types
    import antenv
    from trn_agent_boot.trn_boot import _ntff_profile_via_ctypes
    hook = _ntff_profile_via_ctypes("/opt/axon/libaxon_pjrt.so")
    mod = types.ModuleType("antenv.axon_hooks")
    mod._hook = hook
    mod.get_axon_ntff_profile_hook = lambda: mod._hook
    mod.set_axon_ntff_profile_hook = lambda h: setattr(mod, "_hook", h)
    sys.modules["antenv.axon_hooks"] = mod
    antenv.axon_hooks = mod


def kernel(query, key_padding_mask, attn_mask, Wq, bq, Wk, bk, Wv, bv, Wo, bo,
           _profile=False):
    from concourse.bass_utils import run_bass_kernel_spmd

    if _profile:
        try:
            _ensure_ntff_hook()
        except Exception as e:  # profiling is best-effort
            print(f"ntff hook unavailable: {e}")

    query = np.asarray(query, dtype=np.float32)
    key_padding_mask = np.asarray(key_padding_mask).astype(bool)
    in_maps = make_in_maps(query, key_padding_mask,
                           np.asarray(Wq, np.float32), np.asarray(bq, np.float32),
                           np.asarray(Wk, np.float32),
                           np.asarray(Wv, np.float32),
                           np.asarray(Wo, np.float32))
    nc = _get_nc()
    res = run_bass_kernel_spmd(nc, in_maps, core_ids=list(range(NCORES)),
                               trace=_profile)
    parts = [res.results[c]["outp"] for c in range(NCORES)]
    out = combine_outputs(parts, query, key_padding_mask,
                          np.asarray(Wv, np.float32), np.asarray(bv, np.float32),
                          np.asarray(Wo, np.float32), np.asarray(bo, np.float32))
    if _profile:
        return out, res
    return out
